# revision 7
# baseline (speedup 1.0000x reference)
"""Randomized Hadamard transform kernel for Trainium2 (8 NeuronCores, SPMD).

Math: out = FWHT(x * seed) / sqrt(4096), with the reference's butterfly
ordering equal to the Sylvester Hadamard matrix H_4096 (natural order).
Since H_4096 = H_32 (x) H_128 (Kronecker, hi-major index split
c = hi*128 + lo), each 4096-wide row transform becomes two small
contractions:

    out[r, j*128 + l] = (1/64) * sum_{hi,lo} H32[hi,j] H128[lo,l] x~[r, hi*128+lo]

with x~ = x * seed.

Layout trick: a TensorEngine matmul computes out[m, n] = sum_k lhsT[k, m]
* rhs[k, n] (k = partition dim of both operands). Putting the *data* in
the stationary operand (lhsT) and the small Hadamard factor in the moving
operand (rhs) both contracts the data's partition index AND rotates a
128-wide free window onto the output partitions. Two such passes apply
both factors and return the data to a store-friendly orientation — no
explicit transposes anywhere.

Per 128-row tile (rows r = r0 + r_hi*4 + r_lo, cols c = hi*128 + lo):
  load   Lt[(r_lo,hi), (r_hi,lo)]   <- x    (512B-contiguous DMA chunks)
  mult   Xt = Lt * S_rep            (DVE; S_rep host-replicated seed)
  pass1  chunk r_hi: psum[lo, (r_lo,j)] = sum_(r_lo,hi) Xt[(r_lo,hi), lo] * (I4 (x) H32)
  pass2  chunk r_hi: psum[(r_lo,j), l] = sum_lo W[lo, (r_lo,j)] * (H128/64)
  store  O[(r_lo,j), (r_hi,l)]      -> out  (512B-contiguous DMA chunks)
"""

import numpy as np

import concourse.bass as bass
import concourse.mybir as mybir
from concourse import bacc
import concourse.tile as tile
from concourse.bass_utils import run_bass_kernel_spmd

N_CORES = 8
R_FULL = 8192
C = 4096
R_CORE = R_FULL // N_CORES  # 1024 rows per core
P = 128
NHI, NLO, NRL, NRH = 32, 128, 4, 32  # c = hi*128+lo ; tile rows = r_hi*4+r_lo


def _sylvester(n: int) -> np.ndarray:
    h = np.array([[1.0]], dtype=np.float64)
    while h.shape[0] < n:
        h = np.block([[h, h], [h, -h]])
    return h


def _consts():
    h32 = _sylvester(NHI)
    h128 = _sylvester(NLO)
    k1 = np.kron(np.eye(NRL), h32).astype(np.float32)  # [128,128], contracts (r_lo,hi)
    k2 = (h128 / 64.0).astype(np.float32)  # [128,128], contracts lo, folds scale
    return k1, k2


def build_nc(rows: int = R_CORE):
    """Build the per-core Bass program for `rows` rows (multiple of 128)."""
    assert rows % P == 0
    n_tiles = rows // P

    k1_np, k2_np = _consts()

    nc = bacc.Bacc("TRN2", target_bir_lowering=False, debug=False)
    x_in = nc.dram_tensor("x", [rows, C], mybir.dt.float32, kind="ExternalInput")
    s_in = nc.dram_tensor("srep", [P, C], mybir.dt.float32, kind="ExternalInput")
    y_out = nc.dram_tensor("y", [rows, C], mybir.dt.float32, kind="ExternalOutput")
    k1_dram = nc.inline_tensor(k1_np, "k1")
    k2_dram = nc.inline_tensor(k2_np, "k2")

    f32 = mybir.dt.float32

    with tile.TileContext(nc) as tc:
        with (
            tc.tile_pool(name="consts", bufs=1) as cpool,
            tc.tile_pool(name="lt", bufs=2) as lt_pool,
            tc.tile_pool(name="xt", bufs=2) as xt_pool,
            tc.tile_pool(name="w", bufs=2) as w_pool,
            tc.tile_pool(name="o", bufs=2) as o_pool,
            tc.tile_pool(name="ps1", bufs=3, space="PSUM") as ps1_pool,
            tc.tile_pool(name="ps2", bufs=3, space="PSUM") as ps2_pool,
        ):
            k1 = cpool.tile([P, P], f32)
            k2 = cpool.tile([P, P], f32)
            srep = cpool.tile([P, C], f32)
            nc.sync.dma_start(out=k1[:], in_=k1_dram[:])
            nc.sync.dma_start(out=k2[:], in_=k2_dram[:])
            nc.sync.dma_start(out=srep[:], in_=s_in[:])

            for t in range(n_tiles):
                r0 = t * P
                # ---- load: Lt[(r_lo,hi), (r_hi,lo)] = x[r0 + r_hi*4 + r_lo, hi*128+lo]
                lt = lt_pool.tile([P, C], f32)
                src = x_in[r0 : r0 + P, :].rearrange(
                    "(rh rl) (hi lo) -> rl hi rh lo", rl=NRL, lo=NLO
                )
                nc.sync.dma_start(out=lt[:], in_=src)

                # ---- seed multiply
                xt = xt_pool.tile([P, C], f32)
                nc.vector.tensor_mul(out=xt[:], in0=lt[:], in1=srep[:])

                # ---- pass 1: contract (r_lo,hi) with I4 (x) H32; lo -> partitions
                w = w_pool.tile([P, C], f32)
                for g in range(NRH // 4):
                    ps = ps1_pool.tile([P, 512], f32)
                    for q in range(4):
                        rh = 4 * g + q
                        nc.tensor.matmul(
                            ps[:, q * P : (q + 1) * P],
                            lhsT=xt[:, rh * P : (rh + 1) * P],
                            rhs=k1[:],
                            start=True,
                            stop=True,
                        )
                    if g % 2 == 0:
                        nc.vector.tensor_copy(out=w[:, g * 512 : (g + 1) * 512], in_=ps[:])
                    else:
                        nc.scalar.copy(out=w[:, g * 512 : (g + 1) * 512], in_=ps[:])

                # ---- pass 2: contract lo with H128/64; (r_lo,j) -> partitions
                o = o_pool.tile([P, C], f32)
                for g in range(NRH // 4):
                    ps = ps2_pool.tile([P, 512], f32)
                    for q in range(4):
                        rh = 4 * g + q
                        nc.tensor.matmul(
                            ps[:, q * P : (q + 1) * P],
                            lhsT=w[:, rh * P : (rh + 1) * P],
                            rhs=k2[:],
                            start=True,
                            stop=True,
                        )
                    if g % 2 == 1:
                        nc.vector.tensor_copy(out=o[:, g * 512 : (g + 1) * 512], in_=ps[:])
                    else:
                        nc.scalar.copy(out=o[:, g * 512 : (g + 1) * 512], in_=ps[:])

                # ---- store: O[(r_lo,j), (r_hi,l)] -> y[r0 + r_hi*4 + r_lo, j*128+l]
                dst = y_out[r0 : r0 + P, :].rearrange(
                    "(rh rl) (j l) -> rl j rh l", rl=NRL, l=NLO
                )
                nc.scalar.dma_start(out=dst, in_=o[:])

    nc.compile()
    nc.finalize()
    return nc


_NC_CACHE: dict[int, object] = {}


def _get_nc(rows: int):
    if rows not in _NC_CACHE:
        _NC_CACHE[rows] = build_nc(rows)
    return _NC_CACHE[rows]


def _make_srep(seed: np.ndarray) -> np.ndarray:
    # S_rep[p=(r_lo,hi), f=(r_hi,lo)] = seed[hi*128+lo]
    return np.ascontiguousarray(
        np.tile(seed.reshape(NHI, NLO), (NRL, NRH)).astype(np.float32)
    )


def run(x: np.ndarray, seed: np.ndarray, trace: bool = False):
    """Run on 8 cores; returns (out, BassKernelResults)."""
    x = np.ascontiguousarray(x, dtype=np.float32)
    seed = np.ascontiguousarray(seed, dtype=np.float32)
    nc = _get_nc(R_CORE)
    srep = _make_srep(seed)
    in_maps = [
        {"x": x[i * R_CORE : (i + 1) * R_CORE], "srep": srep} for i in range(N_CORES)
    ]
    res = run_bass_kernel_spmd(nc, in_maps, core_ids=list(range(N_CORES)), trace=trace)
    out = np.concatenate([res.results[i]["y"] for i in range(N_CORES)], axis=0)
    return out, res


def kernel(x: np.ndarray, seed: np.ndarray) -> np.ndarray:
    out, _ = run(x, seed)
    return out


# revision 9
# speedup vs baseline: 1.0980x; 1.0980x over previous
"""Randomized Hadamard transform kernel for Trainium2 (8 NeuronCores, SPMD).

Math: out = FWHT(x * seed) / sqrt(4096); the reference butterfly equals the
Sylvester Hadamard matrix, and H_4096 = H_32 (x) H_128 (c = hi*128 + lo):

    out[r, j*128 + l] = (1/64) * sum_{hi,lo} H32[hi,j] H128[lo,l] x~[r, hi*128+lo]

Layout trick: matmul computes out[m, n] = sum_k lhsT[k, m] * rhs[k, n].
With the *data* as stationary lhsT and the Hadamard factor as moving rhs,
one MM both contracts the data's partition index and rotates a 128-wide
free window onto the output partitions. Two such passes apply both factors
and land in a store-friendly orientation — no explicit transposes.

Per 128-row tile (rows r = r0 + r_hi*4 + r_lo, cols c = hi*128 + lo):
  load   Lt[(r_lo,hi), (r_hi,lo)] <- x   in 4 quarter-DMAs (32-row slabs)
  mult   Xt = Lt * S_rep  per quarter (DVE; S_rep is r_hi-periodic -> [128,1024])
  pass1  chunk r_hi: psum[lo, (r_lo,j)] = sum_(r_lo,hi) Xt[(r_lo,hi), lo] * (I4 (x) H32)
  pass2  chunk r_hi: psum[(r_lo,j), l] = sum_lo W[lo, (r_lo,j)] * (H128/64)
  store  O[(r_lo,j), (r_hi,l)] -> out  in 2 half-DMAs (64-row slabs)
All DMA chunks are 512B-contiguous.
"""

import numpy as np

import concourse.mybir as mybir
from concourse import bacc
import concourse.tile as tile
from concourse.bass_utils import run_bass_kernel_spmd

N_CORES = 8
R_FULL = 8192
C = 4096
R_CORE = R_FULL // N_CORES  # 1024 rows per core
P = 128
NHI, NLO, NRL, NRH = 32, 128, 4, 32  # c = hi*128+lo ; tile rows = r_hi*4+r_lo
QF = 1024  # quarter free-size (8 r_hi chunks)


def _sylvester(n: int) -> np.ndarray:
    h = np.array([[1.0]], dtype=np.float64)
    while h.shape[0] < n:
        h = np.block([[h, h], [h, -h]])
    return h


def _consts():
    k1 = np.kron(np.eye(NRL), _sylvester(NHI)).astype(np.float32)
    k2 = (_sylvester(NLO) / 64.0).astype(np.float32)
    return k1, k2


def build_nc(rows: int = R_CORE):
    assert rows % P == 0
    n_tiles = rows // P

    k1_np, k2_np = _consts()

    nc = bacc.Bacc("TRN2", target_bir_lowering=False, debug=False)
    x_in = nc.dram_tensor("x", [rows, C], mybir.dt.float32, kind="ExternalInput")
    s_in = nc.dram_tensor("srep", [P, QF], mybir.dt.float32, kind="ExternalInput")
    y_out = nc.dram_tensor("y", [rows, C], mybir.dt.float32, kind="ExternalOutput")
    k1_dram = nc.inline_tensor(k1_np, "k1")
    k2_dram = nc.inline_tensor(k2_np, "k2")

    f32 = mybir.dt.float32

    with tile.TileContext(nc) as tc:
        with (
            tc.tile_pool(name="consts", bufs=1) as cpool,
            tc.tile_pool(name="lt", bufs=8) as lt_pool,
            tc.tile_pool(name="xt", bufs=8) as xt_pool,
            tc.tile_pool(name="w", bufs=3) as w_pool,
            tc.tile_pool(name="o", bufs=4) as o_pool,
            tc.tile_pool(name="ps1", bufs=4, space="PSUM") as ps1_pool,
            tc.tile_pool(name="ps2", bufs=4, space="PSUM") as ps2_pool,
        ):
            k1 = cpool.tile([P, P], f32)
            k2 = cpool.tile([P, P], f32)
            srep = cpool.tile([P, QF], f32)
            # constants ride the Scalar HWDGE ring so the first x load
            # starts immediately on the Sync ring
            nc.scalar.dma_start(out=k1[:], in_=k1_dram[:])
            nc.scalar.dma_start(out=k2[:], in_=k2_dram[:])
            nc.scalar.dma_start(out=srep[:], in_=s_in[:])

            for t in range(n_tiles):
                r0 = t * P
                # ---- load + seed multiply, in 4 quarters (32-row slabs)
                xtq = []
                for qi in range(4):
                    ltq = lt_pool.tile([P, QF], f32, tag="ltq")
                    src = x_in[r0 + 32 * qi : r0 + 32 * (qi + 1), :].rearrange(
                        "(rh rl) (hi lo) -> rl hi rh lo", rl=NRL, lo=NLO
                    )
                    nc.sync.dma_start(out=ltq[:], in_=src)
                    xq = xt_pool.tile([P, QF], f32, tag="xtq")
                    nc.vector.tensor_mul(out=xq[:], in0=ltq[:], in1=srep[:])
                    xtq.append(xq)

                # ---- pass 1: contract (r_lo,hi) with I4 (x) H32; lo -> partitions
                w = w_pool.tile([P, C], f32)
                for g in range(NRH // 4):
                    ps = ps1_pool.tile([P, 512], f32)
                    for q in range(4):
                        rh = 4 * g + q
                        nc.tensor.matmul(
                            ps[:, q * P : (q + 1) * P],
                            lhsT=xtq[rh // 8][:, (rh % 8) * P : (rh % 8 + 1) * P],
                            rhs=k1[:],
                            start=True,
                            stop=True,
                        )
                    if g % 2 == 0:
                        nc.vector.tensor_copy(out=w[:, g * 512 : (g + 1) * 512], in_=ps[:])
                    else:
                        nc.scalar.copy(out=w[:, g * 512 : (g + 1) * 512], in_=ps[:])

                # ---- pass 2: contract lo with H128/64; (r_lo,j) -> partitions
                oh = [o_pool.tile([P, 2048], f32, tag="oh", name=f"oh{t}_{i}") for i in range(2)]
                for g in range(NRH // 4):
                    ps = ps2_pool.tile([P, 512], f32)
                    for q in range(4):
                        rh = 4 * g + q
                        nc.tensor.matmul(
                            ps[:, q * P : (q + 1) * P],
                            lhsT=w[:, rh * P : (rh + 1) * P],
                            rhs=k2[:],
                            start=True,
                            stop=True,
                        )
                    dst_sb = oh[g // 4][:, (g % 4) * 512 : (g % 4 + 1) * 512]
                    if g % 2 == 1:
                        nc.vector.tensor_copy(out=dst_sb, in_=ps[:])
                    else:
                        nc.scalar.copy(out=dst_sb, in_=ps[:])
                    # ---- store halves as soon as each is drained
                    if g % 4 == 3:
                        h = g // 4
                        dst = y_out[r0 + 64 * h : r0 + 64 * (h + 1), :].rearrange(
                            "(rh rl) (j l) -> rl j rh l", rl=NRL, l=NLO
                        )
                        nc.scalar.dma_start(out=dst, in_=oh[h][:])

    nc.compile()
    nc.finalize()
    return nc


_NC_CACHE: dict[int, object] = {}


def _get_nc(rows: int):
    if rows not in _NC_CACHE:
        _NC_CACHE[rows] = build_nc(rows)
    return _NC_CACHE[rows]


def _make_srep(seed: np.ndarray) -> np.ndarray:
    # srep[p=(r_lo,hi), f=(r_hi,lo)] = seed[hi*128+lo], r_hi-periodic -> 8 reps
    return np.ascontiguousarray(
        np.tile(seed.reshape(NHI, NLO), (NRL, QF // NLO)).astype(np.float32)
    )


def run(x: np.ndarray, seed: np.ndarray, trace: bool = False):
    x = np.ascontiguousarray(x, dtype=np.float32)
    seed = np.ascontiguousarray(seed, dtype=np.float32)
    nc = _get_nc(R_CORE)
    srep = _make_srep(seed)
    in_maps = [
        {"x": x[i * R_CORE : (i + 1) * R_CORE], "srep": srep} for i in range(N_CORES)
    ]
    res = run_bass_kernel_spmd(nc, in_maps, core_ids=list(range(N_CORES)), trace=trace)
    out = np.concatenate([res.results[i]["y"] for i in range(N_CORES)], axis=0)
    return out, res


def kernel(x: np.ndarray, seed: np.ndarray) -> np.ndarray:
    out, _ = run(x, seed)
    return out
